# revision 15
# baseline (speedup 1.0000x reference)
"""ChessBoardAttention Trainium2 kernel.

Full inputs -> full output. The 32 independent (batch, chessboard-offset)
attention problems are sharded 4-per-core across 8 NeuronCores; the
chessboard gather/scatter is pure data movement done host-side as part of
sharding.

Per-core device kernel, per problem (x_off: [64, 2304] f32):
  q/k = relu(Wqk @ x + b)            [8, L]   (bias via ones-row in x)
  vT  = relu(x_chunk.T @ Wv.T + bv)  [L, 64]  (computed transposed, 128-row chunks)
  S_T[m, l] = k[:,m-chunk].T @ q     scores computed TRANSPOSED so that the
                                     AV contraction runs over PSUM partitions
  P_T = exp(S_T)                     (no max-subtraction needed: s in [0, ~20])
  AV: out[c, l] = sum_m vT_aug[m, c] P_T[m, l], where vT_aug column 64 is
      filled with 1/gamma so row 64 of the accumulator is Z/gamma, making
      the final normalize out * recip(Z/gamma) = gamma * softmax @ v.
  out = AV * recip + x_off           (residual)
"""

import numpy as np

import concourse.bass as bass
import concourse.tile as tile
from concourse import mybir
from concourse.bass_utils import run_bass_kernel_spmd

F32 = mybir.dt.float32
AT = mybir.AluOpType

B, C, H, W = 2, 64, 192, 192
C8 = 8
HQ, WQ = H // 4, W // 4
L = HQ * WQ            # 2304
NPROB = 4              # problems per core
NCORES = 8
NM = L // 128          # 18 m-chunks of 128
LBLOCKS = [(0, 512), (512, 512), (1024, 512), (1536, 512), (2048, 256)]
VS = C + 1             # v-chunk stride in vT_sb (64 channels + 1/gamma col)
SGRP = 3               # m-chunks per score-psum group (3 banks)
NGRP = NM // SGRP      # 6 groups


def split_drain_waits(nc, keep=1):
    """This walrus build rejects instructions carrying more than a couple of
    sem-waits. Move excess waits onto single-wait DRAIN instructions inserted
    just before the offender on the same engine (drains with one wait are
    known-good through codegen)."""
    for f in nc.m.functions:
        for bb in f.blocks:
            insts = bb.instructions
            idx = 0
            while idx < len(insts):
                i = insts[idx]
                si = i.sync_info
                lim = keep
                if si is not None and si.on_wait and len(si.on_wait) > lim:
                    waits = list(si.on_wait)
                    si.on_wait = waits[-lim:]
                    for k, wt in enumerate(waits[:-lim]):
                        d = mybir.InstDrain(
                            name=f"{i.name}_wsplit{k}", ins=[], outs=[],
                            bass_is_fusable=False,
                        )
                        d.engine = i.engine
                        d.sync_info = mybir.SyncInfo(on_wait=[wt], on_update=[])
                        nc.register_instruction(d)
                        insts.insert(idx, d)
                        idx += 1
                idx += 1


def build_module():
    nc = bass.Bass("TRN2", target_bir_lowering=False, debug=False,
                   enable_asserts=False)
    xoffs = nc.dram_tensor("xoffs", [NPROB, C, L], F32, kind="ExternalInput").ap()
    wqk = nc.dram_tensor("wqk", [C + 1, 40], F32, kind="ExternalInput").ap()
    wv = nc.dram_tensor("wv", [C + 1, C], F32, kind="ExternalInput").ap()
    invg_col = nc.dram_tensor("invg_col", [128, NM], F32, kind="ExternalInput").ap()
    out_d = nc.dram_tensor("out", [NPROB, C, L], F32, kind="ExternalOutput").ap()

    with tile.TileContext(nc) as tc:
        with (
            tc.tile_pool(name="singles", bufs=1) as singles,
            tc.tile_pool(name="io", bufs=2) as io,
            tc.tile_pool(name="qk", bufs=2) as qkp,
            tc.tile_pool(name="vt", bufs=2) as vtp,
            tc.tile_pool(name="pt", bufs=2) as ptp,
            tc.tile_pool(name="small", bufs=2) as smallp,
            tc.tile_pool(name="ps_s", bufs=2, space="PSUM") as ps_s_p,
            tc.tile_pool(name="ps_o", bufs=1, space="PSUM") as ps_o_p,
            tc.tile_pool(name="ps_proj", bufs=1, space="PSUM") as ps_proj_p,
            tc.tile_pool(name="dram", bufs=2, space="DRAM") as dramp,
        ):
            wqk_sb = singles.tile([C + 1, 40], F32)
            nc.sync.dma_start(out=wqk_sb, in_=wqk)
            wv_sb = singles.tile([C + 1, C], F32)
            nc.sync.dma_start(out=wv_sb, in_=wv)

            for p in range(NPROB):
                # ---- load x (+ ones row for the bias trick) ----
                x_sb = io.tile([C + 1, L], F32, tag="x")
                nc.sync.dma_start(out=x_sb[0:C, :], in_=xoffs[p])
                nc.gpsimd.memset(x_sb[C : C + 1, :], 1.0)

                # ---- q/k projection: [16, L] = wqk.T @ x_aug, relu ----
                q_sb = qkp.tile([C8, L], F32, tag="q")
                k_sb = qkp.tile([C8, L], F32, tag="k")
                for st, w in LBLOCKS:
                    ps = ps_proj_p.tile([128, 512], F32, tag="proj")
                    nc.tensor.matmul(
                        ps[:40, :w], lhsT=wqk_sb, rhs=x_sb[:, st : st + w],
                        start=True, stop=True,
                    )
                    nc.vector.tensor_scalar_max(
                        out=q_sb[:, st : st + w], in0=ps[0:C8, :w], scalar1=0.0)
                    nc.vector.tensor_scalar_max(
                        out=k_sb[:, st : st + w], in0=ps[32:40, :w], scalar1=0.0)

                # ---- v projection, transposed: vT[m, c] in 128-row chunks ----
                vT_sb = vtp.tile([128, NM * VS], F32, tag="vt")
                vT3 = vT_sb.rearrange("p (n c) -> p n c", c=VS)
                nc.sync.dma_start(out=vT3[:, :, C], in_=invg_col)
                for g in range(3):
                    cnt = 8 if g < 2 else NM - 16
                    ps = ps_proj_p.tile([128, 512], F32, tag="proj")
                    for j in range(cnt):
                        mc = g * 8 + j
                        nc.tensor.matmul(
                            ps[:, j * C : (j + 1) * C],
                            lhsT=x_sb[:, mc * 128 : (mc + 1) * 128],
                            rhs=wv_sb, start=True, stop=True,
                        )
                    ps3 = ps.rearrange("p (n c) -> p n c", c=C)
                    nc.vector.tensor_scalar_max(
                        out=vT3[:, g * 8 : g * 8 + cnt, 0:C],
                        in0=ps3[:, 0:cnt, :], scalar1=0.0)

                # ---- attention over l-blocks ----
                av_sb = io.tile([C + 1, L], F32, tag="av")
                for st, w in LBLOCKS:
                    pT_sb = ptp.tile([128, NM * 512], F32, tag="pt")
                    pT3 = pT_sb.rearrange("p (n c) -> p n c", c=512)
                    for g in range(NGRP):
                        ps_s = ps_s_p.tile([128, SGRP * 512], F32, tag="s")
                        for j in range(SGRP):
                            mc = g * SGRP + j
                            nc.tensor.matmul(
                                ps_s[:, j * 512 : j * 512 + w],
                                lhsT=k_sb[:, mc * 128 : (mc + 1) * 128],
                                rhs=q_sb[:, st : st + w],
                                start=True, stop=True,
                            )
                        ps_s3 = ps_s.rearrange("p (n c) -> p n c", c=512)
                        nc.scalar.activation(
                            out=pT3[:, g * SGRP : (g + 1) * SGRP, :w],
                            in_=ps_s3[:, :, :w],
                            func=mybir.ActivationFunctionType.Exp,
                        )
                    ps_o = ps_o_p.tile([C + 1, 512], F32, tag="o")
                    for mc in range(NM):
                        nc.tensor.matmul(
                            ps_o[:, :w],
                            lhsT=vT3[:, mc, :],
                            rhs=pT3[:, mc, :w],
                            start=(mc == 0), stop=(mc == NM - 1),
                        )
                    nc.vector.tensor_copy(av_sb[:, st : st + w], ps_o[:, :w])

                # ---- normalize (row C of av_sb is Z/gamma), scale, residual ----
                nc.vector.reciprocal(
                    out=av_sb[C : C + 1, :], in_=av_sb[C : C + 1, :])
                dram_rec = dramp.tile([1, L], F32, tag="drec")
                nc.sync.dma_start(out=dram_rec, in_=av_sb[C : C + 1, :])
                rec_rep = smallp.tile([C, L], F32, tag="recrep")
                rec_b = bass.AP(
                    tensor=dram_rec.tensor, offset=dram_rec.offset,
                    ap=[[0, C]] + list(dram_rec.ap)[1:])
                nc.sync.dma_start(out=rec_rep, in_=rec_b)
                nc.vector.tensor_tensor(
                    out=av_sb[0:C, :], in0=av_sb[0:C, :], in1=rec_rep, op=AT.mult)
                nc.gpsimd.tensor_tensor(
                    out=av_sb[0:C, :], in0=av_sb[0:C, :], in1=x_sb[0:C, :], op=AT.add)
                nc.sync.dma_start(out=out_d[p], in_=av_sb[0:C, :])

    split_drain_waits(nc)
    return nc


_NC = None


def _get_nc():
    global _NC
    if _NC is None:
        _NC = build_module()
    return _NC


def make_in_maps(x, Wq, bq, Wk, bk, Wv, bv, gamma):
    x = np.asarray(x, np.float32)
    xoff = (
        x.reshape(B, C, HQ, 4, WQ, 4)
        .transpose(0, 3, 5, 1, 2, 4)
        .reshape(B * 16, C, L)
    )
    wqk = np.zeros((C + 1, 40), np.float32)   # q -> psum parts 0-7, k -> 32-39
    wqk[:C, 0:C8] = np.asarray(Wq).T
    wqk[C, 0:C8] = np.asarray(bq)
    wqk[:C, 32:40] = np.asarray(Wk).T
    wqk[C, 32:40] = np.asarray(bk)
    wv = np.concatenate([np.asarray(Wv).T, np.asarray(bv)[None, :]], 0).astype(
        np.float32
    )                                         # [65, 64]
    with np.errstate(divide="ignore"):
        invg = np.float32(1.0) / np.float32(np.asarray(gamma).reshape(-1)[0])
    invg_col = np.full((128, NM), invg, np.float32)
    in_maps = []
    for c in range(NCORES):
        in_maps.append(
            {
                "xoffs": np.ascontiguousarray(xoff[c * NPROB : (c + 1) * NPROB]),
                "wqk": wqk,
                "wv": wv,
                "invg_col": invg_col,
            }
        )
    return in_maps


def unshard(results):
    outp = np.concatenate([results[c]["out"] for c in range(NCORES)], 0)
    return (
        outp.reshape(B, 4, 4, C, HQ, WQ)
        .transpose(0, 3, 4, 1, 5, 2)
        .reshape(B, C, H, W)
        .astype(np.float32)
    )


def kernel(**inputs):
    nc = _get_nc()
    in_maps = make_in_maps(**inputs)
    res = run_bass_kernel_spmd(nc, in_maps, list(range(NCORES)))
    return unshard(res.results)


# revision 19
# speedup vs baseline: 2.9235x; 2.9235x over previous
"""ChessBoardAttention Trainium2 kernel.

Full inputs -> full output. The 32 independent (batch, chessboard-offset)
attention problems are sharded 4-per-core across 8 NeuronCores; the
chessboard gather/scatter is pure data movement done host-side as part of
sharding.

Per-core device kernel, per problem (x_off: [64, 2304] f32):
  q/k = relu(Wqk @ x + b)            [8, L]   (bias via ones-row in x)
  vT  = relu(x_chunk.T @ Wv.T + bv)  [L, 64]  (computed transposed, 128-row chunks)
  S_T[m, l] = k[:,m-chunk].T @ q     scores computed TRANSPOSED so that the
                                     AV contraction runs over PSUM partitions
  P_T = exp(S_T)                     (no max-subtraction needed: s in [0, ~20])
  AV: out[c, l] = sum_m vT_aug[m, c] P_T[m, l], where vT_aug column 64 is
      filled with 1/gamma so row 64 of the accumulator is Z/gamma, making
      the final normalize out * recip(Z/gamma) = gamma * softmax @ v.
  out = AV * recip + x_off           (residual)
"""

import numpy as np

import concourse.bass as bass
import concourse.tile as tile
from concourse import mybir
from concourse.bass_utils import run_bass_kernel_spmd

F32 = mybir.dt.float32
F32R = mybir.dt.float32r
AT = mybir.AluOpType

B, C, H, W = 2, 64, 192, 192
C8 = 8
HQ, WQ = H // 4, W // 4
L = HQ * WQ            # 2304
NPROB = 4              # problems per core
NCORES = 8
NM = L // 128          # 18 m-chunks of 128
LBLOCKS = [(0, 512), (512, 512), (1024, 512), (1536, 512), (2048, 256)]
VS = C + 1             # v-chunk stride in vT_sb (64 channels + 1/gamma col)
SGRP = 3               # m-chunks per score-psum group (3 banks)
NGRP = NM // SGRP      # 6 groups


def split_drain_waits(nc, keep=1):
    """This walrus build rejects instructions carrying more than a couple of
    sem-waits. Move excess waits onto single-wait DRAIN instructions inserted
    just before the offender on the same engine (drains with one wait are
    known-good through codegen)."""
    for f in nc.m.functions:
        for bb in f.blocks:
            insts = bb.instructions
            idx = 0
            while idx < len(insts):
                i = insts[idx]
                si = i.sync_info
                lim = keep
                if si is not None and si.on_wait and len(si.on_wait) > lim:
                    waits = list(si.on_wait)
                    si.on_wait = waits[-lim:]
                    for k, wt in enumerate(waits[:-lim]):
                        d = mybir.InstDrain(
                            name=f"{i.name}_wsplit{k}", ins=[], outs=[],
                            bass_is_fusable=False,
                        )
                        d.engine = i.engine
                        d.sync_info = mybir.SyncInfo(on_wait=[wt], on_update=[])
                        nc.register_instruction(d)
                        insts.insert(idx, d)
                        idx += 1
                idx += 1


def build_module():
    nc = bass.Bass("TRN2", target_bir_lowering=False, debug=False,
                   enable_asserts=False)
    xoffs = nc.dram_tensor("xoffs", [NPROB, C, L], F32, kind="ExternalInput").ap()
    wqk = nc.dram_tensor("wqk", [C + 1, 40], F32, kind="ExternalInput").ap()
    wv = nc.dram_tensor("wv", [C + 1, C], F32, kind="ExternalInput").ap()
    invg_col = nc.dram_tensor("invg_col", [128, NM], F32, kind="ExternalInput").ap()
    out_d = nc.dram_tensor("out", [NPROB, C, L], F32, kind="ExternalOutput").ap()

    with tile.TileContext(nc) as tc:
        with (
            tc.tile_pool(name="singles", bufs=1) as singles,
            tc.tile_pool(name="io", bufs=2) as io,
            tc.tile_pool(name="qk", bufs=2) as qkp,
            tc.tile_pool(name="vt", bufs=2) as vtp,
            tc.tile_pool(name="pt", bufs=2) as ptp,
            tc.tile_pool(name="small", bufs=2) as smallp,
            tc.tile_pool(name="ps_s", bufs=2, space="PSUM") as ps_s_p,
            tc.tile_pool(name="ps_o", bufs=1, space="PSUM") as ps_o_p,
            tc.tile_pool(name="ps_proj", bufs=1, space="PSUM") as ps_proj_p,
            tc.tile_pool(name="dram", bufs=2, space="DRAM") as dramp,
        ):
            wqk_sb = singles.tile([C + 1, 40], F32)
            nc.sync.dma_start(out=wqk_sb, in_=wqk)
            wv_sb = singles.tile([C + 1, C], F32)
            nc.sync.dma_start(out=wv_sb, in_=wv)

            for p in range(NPROB):
                # ---- load x (+ ones row for the bias trick) ----
                x_sb = io.tile([C + 1, L], F32, tag="x")
                nc.sync.dma_start(out=x_sb[0:C, :], in_=xoffs[p])
                nc.gpsimd.memset(x_sb[C : C + 1, :], 1.0)

                # ---- q/k projection: [16, L] = wqk.T @ x_aug, relu ----
                q_sb = qkp.tile([C8, L], F32R, tag="q")
                k_sb = qkp.tile([C8, L], F32R, tag="k")
                for st, w in LBLOCKS:
                    ps = ps_proj_p.tile([128, 512], F32, tag="proj")
                    nc.tensor.matmul(
                        ps[:40, :w], lhsT=wqk_sb, rhs=x_sb[:, st : st + w],
                        start=True, stop=True,
                    )
                    nc.vector.tensor_scalar_max(
                        out=q_sb[:, st : st + w], in0=ps[0:C8, :w], scalar1=0.0)
                    nc.vector.tensor_scalar_max(
                        out=k_sb[:, st : st + w], in0=ps[32:40, :w], scalar1=0.0)

                # ---- v projection, transposed: vT[m, c] in 128-row chunks ----
                vT_sb = vtp.tile([128, NM * VS], F32R, tag="vt")
                vT3 = vT_sb.rearrange("p (n c) -> p n c", c=VS)
                invg_sb = smallp.tile([128, NM], F32, tag="invg")
                nc.sync.dma_start(out=invg_sb, in_=invg_col)
                nc.vector.tensor_copy(vT3[:, :, C], invg_sb)
                for g in range(3):
                    cnt = 8 if g < 2 else NM - 16
                    ps = ps_proj_p.tile([128, 512], F32, tag="proj")
                    for j in range(cnt):
                        mc = g * 8 + j
                        nc.tensor.matmul(
                            ps[:, j * C : (j + 1) * C],
                            lhsT=x_sb[:, mc * 128 : (mc + 1) * 128],
                            rhs=wv_sb, start=True, stop=True,
                        )
                    ps3 = ps.rearrange("p (n c) -> p n c", c=C)
                    nc.vector.tensor_scalar_max(
                        out=vT3[:, g * 8 : g * 8 + cnt, 0:C],
                        in0=ps3[:, 0:cnt, :], scalar1=0.0)

                # ---- attention over l-blocks ----
                av_sb = io.tile([C + 1, L], F32, tag="av")
                for st, w in LBLOCKS:
                    pT_sb = ptp.tile([128, NM * 512], F32R, tag="pt")
                    pT3 = pT_sb.rearrange("p (n c) -> p n c", c=512)
                    for g in range(NGRP):
                        ps_s = ps_s_p.tile([128, SGRP * 512], F32, tag="s")
                        for j in range(SGRP):
                            mc = g * SGRP + j
                            nc.tensor.matmul(
                                ps_s[:, j * 512 : j * 512 + w],
                                lhsT=k_sb[:, mc * 128 : (mc + 1) * 128],
                                rhs=q_sb[:, st : st + w],
                                start=True, stop=True,
                            )
                        ps_s3 = ps_s.rearrange("p (n c) -> p n c", c=512)
                        nc.scalar.activation(
                            out=pT3[:, g * SGRP : (g + 1) * SGRP, :w],
                            in_=ps_s3[:, :, :w],
                            func=mybir.ActivationFunctionType.Exp,
                        )
                    ps_o = ps_o_p.tile([C + 1, 512], F32, tag="o")
                    for mc in range(NM):
                        nc.tensor.matmul(
                            ps_o[:, :w],
                            lhsT=vT3[:, mc, :],
                            rhs=pT3[:, mc, :w],
                            start=(mc == 0), stop=(mc == NM - 1),
                        )
                    nc.vector.tensor_copy(av_sb[:, st : st + w], ps_o[:, :w])

                # ---- normalize (row C of av_sb is Z/gamma), scale, residual ----
                nc.vector.reciprocal(
                    out=av_sb[C : C + 1, :], in_=av_sb[C : C + 1, :])
                dram_rec = dramp.tile([1, L], F32, tag="drec")
                nc.sync.dma_start(out=dram_rec, in_=av_sb[C : C + 1, :])
                rec_rep = smallp.tile([C, L], F32, tag="recrep")
                rec_b = bass.AP(
                    tensor=dram_rec.tensor, offset=dram_rec.offset,
                    ap=[[0, C]] + list(dram_rec.ap)[1:])
                nc.sync.dma_start(out=rec_rep, in_=rec_b)
                nc.vector.tensor_tensor(
                    out=av_sb[0:C, :], in0=av_sb[0:C, :], in1=rec_rep, op=AT.mult)
                nc.gpsimd.tensor_tensor(
                    out=av_sb[0:C, :], in0=av_sb[0:C, :], in1=x_sb[0:C, :], op=AT.add)
                nc.sync.dma_start(out=out_d[p], in_=av_sb[0:C, :])

    split_drain_waits(nc)
    return nc


_NC = None


def _get_nc():
    global _NC
    if _NC is None:
        _NC = build_module()
    return _NC


def make_in_maps(x, Wq, bq, Wk, bk, Wv, bv, gamma):
    x = np.asarray(x, np.float32)
    xoff = (
        x.reshape(B, C, HQ, 4, WQ, 4)
        .transpose(0, 3, 5, 1, 2, 4)
        .reshape(B * 16, C, L)
    )
    wqk = np.zeros((C + 1, 40), np.float32)   # q -> psum parts 0-7, k -> 32-39
    wqk[:C, 0:C8] = np.asarray(Wq).T
    wqk[C, 0:C8] = np.asarray(bq)
    wqk[:C, 32:40] = np.asarray(Wk).T
    wqk[C, 32:40] = np.asarray(bk)
    wv = np.concatenate([np.asarray(Wv).T, np.asarray(bv)[None, :]], 0).astype(
        np.float32
    )                                         # [65, 64]
    with np.errstate(divide="ignore"):
        invg = np.float32(1.0) / np.float32(np.asarray(gamma).reshape(-1)[0])
    invg_col = np.full((128, NM), invg, np.float32)
    in_maps = []
    for c in range(NCORES):
        in_maps.append(
            {
                "xoffs": np.ascontiguousarray(xoff[c * NPROB : (c + 1) * NPROB]),
                "wqk": wqk,
                "wv": wv,
                "invg_col": invg_col,
            }
        )
    return in_maps


def unshard(results):
    outp = np.concatenate([results[c]["out"] for c in range(NCORES)], 0)
    return (
        outp.reshape(B, 4, 4, C, HQ, WQ)
        .transpose(0, 3, 4, 1, 5, 2)
        .reshape(B, C, H, W)
        .astype(np.float32)
    )


def kernel(**inputs):
    nc = _get_nc()
    in_maps = make_in_maps(**inputs)
    res = run_bass_kernel_spmd(nc, in_maps, list(range(NCORES)))
    return unshard(res.results)
